# revision 26
# baseline (speedup 1.0000x reference)
"""AttentionAggregator (GAT-style message passing) on 8 trn2 NeuronCores via Bass.

Strategy: 1D row partition of destination nodes (adj_rows is sorted, so each
core owns a contiguous edge slice). Each core computes vw_neigh + attention
scores for its own 12500-row shard (dense matmuls), packs them into a bf16
node table [vw(128) | s_n hi/lo bf16 pair], AllGathers the table, then runs
the edge phase: dma_gather of table rows by adj_cols, per-edge softmax
weights, and a one-hot-matmul segment reduction into PSUM. Tiles are split
by 32-row windows (narrow one-hot masks + static PSUM partition offsets)
and 4 col-buckets (int16 gather indices). Self path (vecs @ W0) is fused
into the chunk epilogue.

Numerics: exp() without the segment-max (max edge score ~11 for this problem
family; exp stays finite in f32). Softmax weights and features ride bf16
through the aggregation matmul; scores stay f32 via a hi/lo bf16 pair.
"""

import os
import numpy as np
import ml_dtypes

NCORES = 8
N, E, DIN, DOUT = 100000, 1600000, 256, 128
RSH = N // NCORES            # 12500 rows per core
NCH = 98                     # chunks of 128 rows
RPAD = NCH * 128             # 12544 padded rows per core
NPAD = NCORES * RPAD         # 100352 padded table rows
NBUCK = 4
BUCK = NPAD // NBUCK         # 25088 (< 32768 -> int16 indices)
NWIN = 4                     # 32-row windows per chunk
W = 128 // NWIN              # 32
ELEM = 256                   # table row: 256 bf16 = 512 bytes
SUPER = 2                    # chunks per superchunk (gather granularity)
HALF = RPAD // 2             # 6272-row half-table collectives
NSUP = NCH // SUPER
BF16 = ml_dtypes.bfloat16


def _host_prep(adj_rows, adj_cols, adj_vals, pad_valid=False):
    """Shard + tile the edge list. Tile order: superchunk -> bucket ->
    chunk -> window -> tile. Uniform tile counts across cores.
    pad_valid=True replaces negative (skipped) pad indices with 0 -- needed
    for the simulator, slower on HW (pads fetch real rows)."""
    bounds = np.searchsorted(adj_rows, np.arange(0, N + 1, RSH))
    cores = []
    for c in range(NCORES):
        s, t = bounds[c], bounds[c + 1]
        rows_l = adj_rows[s:t] - c * RSH
        cols_g = adj_cols[s:t]
        vals = adj_vals[s:t]
        colpad = (cols_g // RSH) * RPAD + (cols_g % RSH)
        buck = colpad // BUCK
        col_loc = (colpad % BUCK).astype(np.int64)
        cores.append((rows_l, col_loc, buck, vals))

    # per (chunk, window, bucket) edge lists; uniform tile counts T[j,q,b]
    per_grp = [dict() for _ in range(NCORES)]
    T = np.zeros((NCH, NWIN, NBUCK), np.int64)
    for c in range(NCORES):
        rows_l = cores[c][0]
        wb = np.searchsorted(rows_l, np.arange(0, NCH * 128 + 1, W))
        for j in range(NCH):
            for q in range(NWIN):
                e0, e1 = wb[j * NWIN + q], wb[j * NWIN + q + 1]
                bsl = cores[c][2][e0:e1]
                for b in range(NBUCK):
                    idx = e0 + np.nonzero(bsl == b)[0]
                    per_grp[c][(j, q, b)] = idx
                    T[j, q, b] = max(T[j, q, b], (len(idx) + 127) // 128)

    # slot order: s -> b -> j -> w -> t
    slot_of = {}
    q_ = 0
    sup_b_slots = np.zeros((NSUP, NBUCK), np.int64)
    for s in range(NSUP):
        for b in range(NBUCK):
            for j in range(SUPER * s, SUPER * s + SUPER):
                for w in range(NWIN):
                    slot_of[(j, w, b)] = q_
                    q_ += T[j, w, b]
            sup_b_slots[s, b] = sum(
                T[j, w, b] for j in range(SUPER * s, SUPER * s + SUPER)
                for w in range(NWIN))
    K_tot = q_

    per_core = []
    for c in range(NCORES):
        rows_l, col_loc, _, vals = cores[c]
        idxs = np.full((128, K_tot), -1, np.int64)
        rows_mw = np.full((128, K_tot), -1.0, np.float32)
        vals_a = np.ones((128, K_tot), np.float32)
        for (j, w, b), el in per_grp[c].items():
            Tg = T[j, w, b]
            if Tg == 0:
                continue
            n = len(el)
            q0 = slot_of[(j, w, b)]
            flat = np.full(Tg * 128, 0 if pad_valid else -1, np.int64)
            flat[:n] = col_loc[el]
            r = np.full(Tg * 128, -1.0, np.float32)
            r[:n] = (rows_l[el] - 128 * j - W * w).astype(np.float32)
            v = np.ones(Tg * 128, np.float32)
            v[:n] = vals[el]
            idxs[:, q0:q0 + Tg] = flat.reshape(Tg, 128).T
            rows_mw[:, q0:q0 + Tg] = r.reshape(Tg, 128).T
            vals_a[:, q0:q0 + Tg] = v.reshape(Tg, 128).T

        # index stream per (s, b): i at [i % 16, i // 16], replicated x8
        idx16 = np.zeros((128, K_tot * 8), np.int16)
        nvalid = np.zeros((NSUP, NBUCK), np.int64)
        for s in range(NSUP):
            for b in range(NBUCK):
                ns = int(sup_b_slots[s, b]) * 128
                if ns == 0:
                    continue
                q0 = slot_of[(SUPER * s, 0, b)]
                stream = idxs[:, q0:q0 + ns // 128].T.reshape(-1)
                nvalid[s, b] = int((stream >= 0).sum())
                wv = stream.reshape(ns // 16, 16).T.astype(np.int16)
                c0 = q0 * 8
                for g in range(8):
                    idx16[g * 16:(g + 1) * 16, c0:c0 + ns // 16] = wv

        deg = np.zeros((128, NCH), np.float32)
        cnt = np.bincount(rows_l, minlength=RPAD).astype(np.float32)
        deg[:, :] = cnt.reshape(NCH, 128).T
        per_core.append(dict(idx16=idx16, idxs=idxs,
                             rows_mw=rows_mw.astype(BF16),
                             vals=vals_a, deg=deg, nvalid=nvalid))

    # num_idxs_reg is baked into the program (same for all cores): use the
    # max over cores, and flip pad indices to 0 (valid row) on cores with
    # fewer real edges so every core generates exactly nv_max descriptors.
    nv_max = np.zeros((NSUP, NBUCK), np.int64)
    for c in range(NCORES):
        nv_max = np.maximum(nv_max, per_core[c]["nvalid"])
    for c in range(NCORES):
        pc = per_core[c]
        for s in range(NSUP):
            for b in range(NBUCK):
                ns = int(sup_b_slots[s, b]) * 128
                if ns == 0:
                    continue
                need = int(nv_max[s, b] - pc["nvalid"][s, b])
                if need == 0:
                    continue
                q0 = slot_of[(SUPER * s, 0, b)]
                blk = pc["idx16"][:, q0 * 8:q0 * 8 + ns // 16]
                st = blk[0:16, :].T.reshape(-1).copy()
                neg = np.nonzero(st < 0)[0][:need]
                st[neg] = 0
                wv = st.reshape(ns // 16, 16).T
                for g in range(8):
                    blk[g * 16:(g + 1) * 16, :] = wv
        del pc["idxs"], pc["nvalid"]
    return per_core, T, slot_of, sup_b_slots, nv_max, K_tot


def _build_nc(T, slot_of, sup_b_slots, nv_max, K_tot):
    import concourse.bass as bass
    import concourse.bacc as bacc
    import concourse.mybir as mybir
    import concourse.tile as tile
    from contextlib import ExitStack

    f32 = mybir.dt.float32
    bf16 = mybir.dt.bfloat16
    i32 = mybir.dt.int32
    i16 = mybir.dt.int16
    AluOp = mybir.AluOpType
    Act = mybir.ActivationFunctionType

    nc = bacc.Bacc("TRN2", target_bir_lowering=False, debug=False,
                   num_devices=NCORES)
    vecsT_a = nc.declare_dram_parameter("vecsT_a", [128, RPAD], f32, isOutput=False)
    vecsT_b = nc.declare_dram_parameter("vecsT_b", [128, RPAD], f32, isOutput=False)
    W1sb_in = nc.declare_dram_parameter("W1sb", [128, 256], f32, isOutput=False)
    W0sb_in = nc.declare_dram_parameter("W0sb", [128, 256], f32, isOutput=False)
    W1T_in = nc.declare_dram_parameter("W1T", [128, 256], f32, isOutput=False)
    att_in = nc.declare_dram_parameter("att", [128, 2], f32, isOutput=False)
    attb_in = nc.declare_dram_parameter("attb", [1, 2], f32, isOutput=False)
    b_in = nc.declare_dram_parameter("bvec", [1, 256], f32, isOutput=False)
    idx_in = nc.declare_dram_parameter("idx16", [128, K_tot * 8], i16, isOutput=False)
    rows_in = nc.declare_dram_parameter("rows_mw", [128, K_tot], bf16, isOutput=False)
    vals_in = nc.declare_dram_parameter("vals", [128, K_tot], f32, isOutput=False)
    deg_in = nc.declare_dram_parameter("deg", [128, NCH], f32, isOutput=False)
    ret = nc.declare_dram_parameter("ret", [RPAD, 128], f32, isOutput=True)

    tab_own = nc.dram_tensor("tab_own", [RPAD, ELEM], bf16)
    tab_full = nc.dram_tensor("tab_full", [NPAD, ELEM], bf16, addr_space="Shared")
    ssflat_d = nc.dram_tensor("ssflat", [1, RPAD], f32)

    Kmax_s = max(int(sup_b_slots[s, :].sum()) for s in range(NSUP))

    with tile.TileContext(nc) as tc, ExitStack() as ctx:
        cst = ctx.enter_context(tc.tile_pool(name="cst", bufs=1))
        dns = ctx.enter_context(tc.tile_pool(name="dns", bufs=2))
        dps = ctx.enter_context(tc.tile_pool(name="dps", bufs=2, space="PSUM"))
        dp1 = ctx.enter_context(tc.tile_pool(name="dp1", bufs=1, space="PSUM"))
        gp = ctx.enter_context(tc.tile_pool(name="gp", bufs=2))
        mp = ctx.enter_context(tc.tile_pool(name="mp", bufs=2))
        sp = ctx.enter_context(tc.tile_pool(name="sp", bufs=2))
        eps_ = ctx.enter_context(tc.tile_pool(name="eps", bufs=2))
        cps = ctx.enter_context(tc.tile_pool(name="cps", bufs=2, space="PSUM"))

        # ---------- constants ----------
        io_i = cst.tile([128, 128], i32)
        nc.gpsimd.iota(io_i[:], pattern=[[1, 128]], base=0, channel_multiplier=0)
        iota_bf = cst.tile([128, 128], bf16)
        nc.vector.tensor_copy(iota_bf[:], io_i[:])
        ones1 = cst.tile([1, 128], f32)
        nc.gpsimd.memset(ones1[:], 1.0)
        zt = cst.tile([128, W], bf16)
        nc.gpsimd.memset(zt[:], 0.0)

        W1sb = cst.tile([128, 256], f32)
        nc.sync.dma_start(W1sb[:], W1sb_in[:, :])
        W0sb = cst.tile([128, 256], f32)
        nc.sync.dma_start(W0sb[:], W0sb_in[:, :])
        W1T = cst.tile([128, 256], f32)
        nc.sync.dma_start(W1T[:], W1T_in[:, :])
        att = cst.tile([128, 2], f32)
        nc.sync.dma_start(att[:], att_in[:, :])
        attb = cst.tile([1, 2], f32)
        nc.sync.dma_start(attb[:], attb_in[:, :])
        bsb = cst.tile([1, 256], f32)
        nc.sync.dma_start(bsb[:], b_in[:, :])

        pcc = dp1.tile([128, 4], f32, tag="ps")
        nc.tensor.matmul(pcc[:, 0:2], W1T[:, 0:128], att[:], start=True, stop=True)
        nc.tensor.matmul(pcc[:, 2:4], W1T[:, 128:256], att[:], start=True, stop=True)
        CC = cst.tile([128, 4], f32)
        nc.vector.tensor_copy(CC[:], pcc[:])

        prep = dps.tile([128, 256], f32, tag="pn")
        nc.tensor.matmul(prep[:, :], ones1[:], bsb[:], start=True, stop=True)
        brep = cst.tile([128, 256], f32)
        nc.vector.tensor_copy(brep[:], prep[:])
        pab = dp1.tile([128, 2], f32, tag="pv")
        nc.tensor.matmul(pab[:, :], ones1[:], attb[:], start=True, stop=True)
        attb_rep = cst.tile([128, 2], f32)
        nc.vector.tensor_copy(attb_rep[:], pab[:])

        rows_mw = cst.tile([128, K_tot], bf16)
        nc.sync.dma_start(rows_mw[:], rows_in[:, :])
        vals = cst.tile([128, K_tot], f32)
        nc.sync.dma_start(vals[:], vals_in[:, :])
        deg = cst.tile([128, NCH], f32)
        nc.sync.dma_start(deg[:], deg_in[:, :])

        vw_self = cst.tile([128, RPAD], f32)

        # ---------- dense phase (2 chunks per superchunk, batched DMAs) ----
        for s in range(NSUP):
            A0 = dns.tile([128, 256], f32, tag="A0")
            nc.sync.dma_start(A0[:], vecsT_a[:, 256 * s:256 * s + 256])
            A1 = dns.tile([128, 256], f32, tag="A1")
            nc.sync.dma_start(A1[:], vecsT_b[:, 256 * s:256 * s + 256])
            stg = dns.tile([128, 512], bf16, tag="stg")
            nc.gpsimd.memset(stg[:, :], 0.0)
            ssb2 = dns.tile([128, 4], f32, tag="ssb2")
            for u_ in range(SUPER):
                j = SUPER * s + u_
                a0 = A0[:, 128 * u_:128 * u_ + 128]
                a1 = A1[:, 128 * u_:128 * u_ + 128]
                pn = dps.tile([128, 128], f32, tag="pn")
                ps_ = dp1.tile([128, 2], f32, tag="ps")
                pv = dp1.tile([128, 128], f32, tag="pv")
                nc.tensor.matmul(pn[:], a0, W1sb[:, 0:128], start=True, stop=False)
                nc.tensor.matmul(pn[:], a1, W1sb[:, 128:256], start=False, stop=True)
                nc.tensor.matmul(ps_[:], a0, CC[:, 0:2], start=True, stop=False)
                nc.tensor.matmul(ps_[:], a1, CC[:, 2:4], start=False, stop=True)
                nc.tensor.matmul(pv[:], a0, W0sb[:, 0:128], start=True, stop=False)
                nc.tensor.matmul(pv[:], a1, W0sb[:, 128:256], start=False, stop=True)

                ssb = ssb2[:, 2 * u_:2 * u_ + 2]
                nc.vector.tensor_tensor(out=ssb, in0=ps_[:], in1=attb_rep[:],
                                        op=AluOp.add)
                so = 256 * u_
                nc.vector.tensor_copy(stg[:, so:so + 128], pn[:])
                nc.vector.tensor_copy(stg[:, so + 128:so + 129], ssb[:, 0:1])
                hi_f = dns.tile([128, 1], f32, tag="hi_f")
                nc.vector.tensor_copy(hi_f[:], stg[:, so + 128:so + 129])
                lo_f = dns.tile([128, 1], f32, tag="lo_f")
                nc.vector.tensor_tensor(out=lo_f[:], in0=ssb[:, 0:1],
                                        in1=hi_f[:], op=AluOp.subtract)
                nc.vector.tensor_copy(stg[:, so + 129:so + 130], lo_f[:])
                nc.vector.tensor_copy(vw_self[:, 128 * j:128 * j + 128], pv[:])
            nc.sync.dma_start(
                ssflat_d[0:1, 256 * s:256 * s + 256].rearrange(
                    "one (c p) -> one p c", c=SUPER),
                ssb2[:, :].rearrange("p (c two) -> p c two", two=2)[:, :, 1:2])
            nc.sync.dma_start(
                tab_own[256 * s:256 * s + 256, :].rearrange(
                    "(c p) e -> p c e", c=SUPER),
                stg[:, :].rearrange("p (c e) -> p c e", c=SUPER))

        # ---------- allgather the table ----------
        nc.gpsimd.collective_compute(
            "AllGather", mybir.AluOpType.bypass,
            replica_groups=[list(range(NCORES))],
            ins=[tab_own[:]], outs=[tab_full[:]],
        )

        # ---------- edge phase ----------
        for s in range(NSUP):
            o_s = slot_of[(SUPER * s, 0, 0)]
            K_s = int(sup_b_slots[s, :].sum())
            if K_s == 0:
                continue
            G = gp.tile([128, Kmax_s * ELEM], bf16, tag="G")
            if s < 2:
                nc.gpsimd.memset(G[:, :], 0.0)
            it = gp.tile([128, Kmax_s * 8], i16, tag="it")
            nc.sync.dma_start(it[:, 0:K_s * 8],
                              idx_in[:, o_s * 8:(o_s + K_s) * 8])
            for b in range(NBUCK):
                nsl = int(sup_b_slots[s, b])
                if nsl == 0:
                    continue
                q0 = slot_of[(SUPER * s, 0, b)]
                loc = q0 - o_s
                nc.gpsimd.dma_gather(
                    out_ap=G[:, loc * ELEM:(loc + nsl) * ELEM].rearrange(
                        "p (s e) -> p s e", e=ELEM),
                    in_ap=tab_full[b * BUCK:(b + 1) * BUCK, :],
                    idxs_ap=it[:, loc * 8:(loc + nsl) * 8],
                    num_idxs=nsl * 128,
                    num_idxs_reg=int(nv_max[s, b]),
                    elem_size=ELEM,
                    single_packet=False,
                )

            ssrow = sp.tile([1, 256], f32, tag="ssrow")
            nc.sync.dma_start(ssrow[0:1, :],
                              ssflat_d[0:1, 256 * s:256 * s + 256])
            ssrep = sp.tile([128, 256], f32, tag="ssrep")
            nc.gpsimd.partition_broadcast(ssrep[:], ssrow[0:1, :])

            m = mp.tile([128, Kmax_s * W], bf16, tag="m")
            sse = sp.tile([128, Kmax_s], f32, tag="sse")
            for b in range(NBUCK):
                for j in range(SUPER * s, SUPER * s + SUPER):
                    q0 = slot_of[(j, 0, b)]
                    Tjb = sum(int(T[j, w, b]) for w in range(NWIN))
                    if Tjb == 0:
                        continue
                    loc = q0 - o_s
                    mv = m[:, loc * W:(loc + Tjb) * W].rearrange(
                        "p (k f) -> p k f", f=W)
                    nc.vector.tensor_tensor(
                        out=mv,
                        in0=rows_mw[:, q0:q0 + Tjb].rearrange(
                            "p (k one) -> p k one", one=1
                        ).to_broadcast([128, Tjb, W]),
                        in1=iota_bf[:, 0:W].rearrange(
                            "p (one f) -> p one f", one=1
                        ).to_broadcast([128, Tjb, W]),
                        op=AluOp.is_equal,
                    )
                    for w in range(NWIN):
                        Tg = int(T[j, w, b])
                        if Tg == 0:
                            continue
                        lw = slot_of[(j, w, b)] - o_s
                        col = 128 * (j - SUPER * s) + W * w
                        s2 = sp.tile([128, 24 * W], f32, tag="s2")
                        s2v = s2[:, 0:Tg * W].rearrange("p (k f) -> p k f", f=W)
                        nc.vector.tensor_tensor(
                            out=s2v,
                            in0=m[:, lw * W:(lw + Tg) * W].rearrange(
                                "p (k f) -> p k f", f=W),
                            in1=ssrep[:, col:col + W].rearrange(
                                "p (one f) -> p one f", one=1
                            ).to_broadcast([128, Tg, W]),
                            op=AluOp.mult,
                        )
                        nc.vector.tensor_reduce(
                            out=sse[:, lw:lw + Tg].rearrange(
                                "p (k one) -> p k one", one=1),
                            in_=s2v, op=AluOp.add, axis=mybir.AxisListType.X,
                        )

            # scores, batched over the whole superchunk
            Gv = G[:, 0:K_s * ELEM].rearrange("p (k e) -> p k e", e=ELEM)
            t1 = sp.tile([128, Kmax_s], f32, tag="t1")
            nc.vector.tensor_tensor(
                out=t1[:, 0:K_s].rearrange("p (k one) -> p k one", one=1),
                in0=Gv[:, :, 128:129], in1=Gv[:, :, 129:130], op=AluOp.add)
            t2 = sp.tile([128, Kmax_s], f32, tag="t2")
            nc.vector.tensor_tensor(out=t2[:, 0:K_s], in0=t1[:, 0:K_s],
                                    in1=sse[:, 0:K_s], op=AluOp.add)
            lr = sp.tile([128, Kmax_s], f32, tag="lr")
            nc.vector.tensor_scalar(out=lr[:, 0:K_s], in0=t2[:, 0:K_s],
                                    scalar1=0.2, scalar2=None, op0=AluOp.mult)
            nc.vector.tensor_tensor(out=lr[:, 0:K_s], in0=lr[:, 0:K_s],
                                    in1=t2[:, 0:K_s], op=AluOp.max)
            ex = sp.tile([128, Kmax_s], f32, tag="ex")
            nc.scalar.activation(ex[:, 0:K_s], lr[:, 0:K_s], Act.Exp)
            u = sp.tile([128, Kmax_s], f32, tag="u")
            nc.vector.tensor_tensor(out=u[:, 0:K_s], in0=ex[:, 0:K_s],
                                    in1=vals[:, o_s:o_s + K_s], op=AluOp.mult)
            ub = sp.tile([128, Kmax_s], bf16, tag="ub")
            nc.vector.tensor_copy(ub[:, 0:K_s], u[:, 0:K_s])
            iv = sp.tile([128, Kmax_s], f32, tag="iv")
            nc.vector.reciprocal(iv[:, 0:K_s], vals[:, o_s:o_s + K_s])
            ivb = sp.tile([128, Kmax_s], bf16, tag="ivb")
            nc.vector.tensor_copy(ivb[:, 0:K_s], iv[:, 0:K_s])
            nc.vector.tensor_copy(
                Gv[:, :, 130:131],
                ivb[:, 0:K_s].rearrange("p (k one) -> p k one", one=1))

            wm = mp.tile([128, Kmax_s * W], bf16, tag="wm")
            nc.vector.tensor_tensor(
                out=wm[:, 0:K_s * W].rearrange("p (k f) -> p k f", f=W),
                in0=m[:, 0:K_s * W].rearrange("p (k f) -> p k f", f=W),
                in1=ub[:, 0:K_s].rearrange(
                    "p (k one) -> p k one", one=1).to_broadcast([128, K_s, W]),
                op=AluOp.mult,
            )

            # per-chunk aggregation + epilogue (psum split in 2 x 64 rows
            # because matmul outputs only allow partition bases 0/32/64)
            for j in range(SUPER * s, SUPER * s + SUPER):
                pcA = cps.tile([64, 131], f32, tag="pcA")
                pcB = cps.tile([64, 131], f32, tag="pcB")
                pcs = [pcA, pcB]
                for w in range(NWIN):
                    pc = pcs[w // 2]
                    base = W * (w % 2)
                    wslots = []
                    for b in range(NBUCK):
                        q0 = slot_of[(j, w, b)]
                        wslots += list(range(q0 - o_s, q0 - o_s + int(T[j, w, b])))
                    if not wslots:
                        nc.tensor.matmul(pc[base:base + W, 0:131], zt[:],
                                         G[:, 0:131], start=True, stop=True)
                        continue
                    for i, qq in enumerate(wslots):
                        nc.tensor.matmul(
                            pc[base:base + W, 0:131],
                            wm[:, qq * W:(qq + 1) * W],
                            G[:, qq * ELEM:qq * ELEM + 131],
                            start=(i == 0), stop=(i == len(wslots) - 1))
                ob = eps_.tile([128, 128], f32, tag="ob")
                dn = eps_.tile([128, 1], f32, tag="dn")
                rc = eps_.tile([128, 1], f32, tag="rc")
                sc = eps_.tile([128, 1], f32, tag="sc")
                msg = eps_.tile([128, 128], f32, tag="msg")
                a1_ = eps_.tile([128, 128], f32, tag="a1")
                r1 = eps_.tile([128, 128], f32, tag="r1")
                a2 = eps_.tile([128, 128], f32, tag="a2")
                r2 = eps_.tile([128, 128], f32, tag="r2")
                for h in range(2):
                    pc = pcs[h]
                    hs = slice(64 * h, 64 * h + 64)
                    nc.vector.tensor_scalar(out=dn[hs, :], in0=pc[:, 130:131],
                                            scalar1=1e-30, scalar2=None,
                                            op0=AluOp.add)
                    nc.vector.reciprocal(rc[hs, :], dn[hs, :])
                    nc.vector.tensor_tensor(out=sc[hs, :], in0=rc[hs, :],
                                            in1=deg[hs, j:j + 1],
                                            op=AluOp.mult)
                    nc.vector.tensor_scalar(out=msg[hs, :], in0=pc[:, 0:128],
                                            scalar1=sc[hs, 0:1], scalar2=None,
                                            op0=AluOp.mult)
                    nc.vector.tensor_tensor(out=a1_[hs, :], in0=msg[hs, :],
                                            in1=brep[hs, 0:128],
                                            op=AluOp.add)
                    nc.scalar.activation(r1[hs, :], a1_[hs, :], Act.Relu)
                    nc.vector.tensor_tensor(
                        out=a2[hs, :],
                        in0=vw_self[hs, 128 * j:128 * j + 128],
                        in1=brep[hs, 128:256], op=AluOp.add)
                    nc.scalar.activation(r2[hs, :], a2[hs, :], Act.Relu)
                    nc.vector.tensor_tensor(out=ob[hs, :], in0=r1[hs, :],
                                            in1=r2[hs, :], op=AluOp.add)
                nc.sync.dma_start(ret[128 * j:128 * j + 128, :], ob[:])

    nc.finalize()
    return nc


_CACHE = {}
LAST_EXEC_NS = None
LAST_PREP_NS = None


def kernel(**inputs) -> np.ndarray:
    vecs = np.ascontiguousarray(np.asarray(inputs["vecs"], np.float32))
    adj_vals = np.asarray(inputs["adj_vals"], np.float32)
    adj_rows = np.asarray(inputs["adj_rows"], np.int64)
    adj_cols = np.asarray(inputs["adj_cols"], np.int64)
    W0 = np.asarray(inputs["W0"], np.float32)
    W1 = np.asarray(inputs["W1"], np.float32)
    b0 = np.asarray(inputs["b0"], np.float32)
    b1 = np.asarray(inputs["b1"], np.float32)
    att0 = np.asarray(inputs["att0"], np.float32)
    att1 = np.asarray(inputs["att1"], np.float32)
    att_b0 = np.asarray(inputs["att_b0"], np.float32)
    att_b1 = np.asarray(inputs["att_b1"], np.float32)

    from concourse.bass_utils import run_bass_kernel_spmd

    per_core, T, slot_of, sup_b_slots, nv_max, K_tot = _host_prep(
        adj_rows, adj_cols, adj_vals, pad_valid=True)

    key = ("nc", K_tot, tuple(T.reshape(-1)), tuple(nv_max.reshape(-1)))
    if key not in _CACHE:
        _CACHE[key] = _build_nc(T, slot_of, sup_b_slots, nv_max, K_tot)
    nc = _CACHE[key]

    W1sb = np.hstack([W1[0:128, :], W1[128:256, :]]).astype(np.float32)
    W0sb = np.hstack([W0[0:128, :], W0[128:256, :]]).astype(np.float32)
    W1T = np.ascontiguousarray(W1.T).astype(np.float32)
    att = np.stack([att1, att0], axis=1).astype(np.float32)
    attb = np.array([[att_b1[0], att_b0[0]]], np.float32)
    bvec = np.concatenate([b1, b0])[None, :].astype(np.float32)

    in_maps = []
    for c in range(NCORES):
        vpad = np.zeros((RPAD, DIN), np.float32)
        vpad[:RSH] = vecs[c * RSH:(c + 1) * RSH]
        vT = np.ascontiguousarray(vpad.T)
        pc = per_core[c]
        in_maps.append({
            "vecsT_a": vT[0:128], "vecsT_b": vT[128:256],
            "W1sb": W1sb, "W0sb": W0sb, "W1T": W1T,
            "att": att, "attb": attb, "bvec": bvec,
            "idx16": pc["idx16"], "rows_mw": pc["rows_mw"],
            "vals": pc["vals"], "deg": pc["deg"],
        })

    global LAST_EXEC_NS
    import time as _time
    _t0 = _time.perf_counter()
    res = run_bass_kernel_spmd(nc, in_maps, core_ids=list(range(NCORES)))
    LAST_EXEC_NS = int((_time.perf_counter() - _t0) * 1e9)
    out = np.concatenate([res.results[c]["ret"][:RSH] for c in range(NCORES)], 0)
    return out.astype(np.float32)
